# revision 2
# baseline (speedup 1.0000x reference)
"""Trainium2 Bass kernel for nn_BeliefPlausibilityFocused.

reference():
    cardinal_fod = inputs.shape[-1] - 1 = 3; n_sets = 8
    bel[..., j] = 1.0 if (j & focal) == focal else 0.0
    pl[...,  j] = 1.0 if (j & focal) >  0    else 0.0
Both outputs are per-pixel broadcast constants of shape
inputs.shape[:-1] + (8,) = [8, 384, 1248, 8]; the input VALUES are unused.

Strategy (pure data-parallel over batch, per sharding hint):
  - 8 cores, one batch element each. Per-core output: bel/pl each
    [384, 1248, 8] f32 = 15.3 MB -> 30.7 MB HBM writes per core.
  - Host computes the two 8-float masks from `focal` and passes one tiny
    [128, 16] pattern tensor. Device replicates it along the free dim with
    log-doubling DVE copies into two SBUF tiles, then issues large HWDGE
    DMA stores to fill the outputs. Memory-roofline ~= 30.7MB / ~358 GB/s.
"""

import numpy as np

import concourse.bacc as bacc
import concourse.mybir as mybir
import concourse.tile as tile
from concourse.bass_utils import run_bass_kernel_spmd

# Problem shapes (hardcoded per contract: kernel.py must be self-contained).
B, H, W, C = 8, 384, 1248, 4
NSETS = 1 << (C - 1)          # 8
N_CORES = 8
P = 128                        # SBUF partitions

PIX = H * W                    # 479232 pixels per batch element
PER_OUT = PIX * NSETS          # 3,833,856 f32 per output per core
PER_PART = PER_OUT // P        # 29,952 f32 per partition
TILE_F = PER_PART // 2         # 14,976 f32 -> SBUF tile 59,904 B/partition

assert PER_OUT % P == 0 and PER_PART % NSETS == 0 and TILE_F % NSETS == 0

_NC_CACHE = {}
LAST_RESULTS = None  # BassKernelResults of the most recent run (for test.py)


def _build_nc():
    """One SPMD program: load 16-float pattern row, replicate, store outputs."""
    nc = bacc.Bacc(None, target_bir_lowering=False)

    pat = nc.dram_tensor("pat", [P, 2 * NSETS], mybir.dt.float32,
                         kind="ExternalInput")
    bel = nc.dram_tensor("bel", [P, PER_PART], mybir.dt.float32,
                         kind="ExternalOutput")
    pl = nc.dram_tensor("pl", [P, PER_PART], mybir.dt.float32,
                        kind="ExternalOutput")

    with tile.TileContext(nc) as tc:
        with tc.tile_pool(name="sbuf", bufs=1) as pool:
            ptile = pool.tile([P, 2 * NSETS], mybir.dt.float32, tag="pt")
            nc.sync.dma_start(out=ptile[:], in_=pat[:])

            belt = pool.tile([P, TILE_F], mybir.dt.float32, tag="belt")
            plt = pool.tile([P, TILE_F], mybir.dt.float32, tag="plt")

            # Fill each tile with the 8-periodic mask via log-doubling copies.
            for t, off in ((belt, 0), (plt, NSETS)):
                nc.vector.tensor_copy(out=t[:, 0:NSETS],
                                      in_=ptile[:, off:off + NSETS])
                f = NSETS
                while f < TILE_F:
                    n = min(f, TILE_F - f)
                    nc.vector.tensor_copy(out=t[:, f:f + n], in_=t[:, 0:n])
                    f += n

            # Each output = 2 stores of [128, TILE_F] (7.67 MB each).
            for r in range(2):
                sl = slice(r * TILE_F, (r + 1) * TILE_F)
                nc.sync.dma_start(out=bel[:, sl], in_=belt[:])
                nc.sync.dma_start(out=pl[:, sl], in_=plt[:])

    nc.finalize()
    return nc


def _get_nc():
    if "nc" not in _NC_CACHE:
        _NC_CACHE["nc"] = _build_nc()
    return _NC_CACHE["nc"]


def kernel(inputs, focal):
    global LAST_RESULTS
    inputs = np.asarray(inputs)
    focal_i = int(np.asarray(focal))
    assert inputs.shape == (B, H, W, C), inputs.shape

    # Host-side mask computation (cheap: 8 elements).
    j = np.arange(NSETS, dtype=np.int64)
    contain = j & focal_i
    bel_mask = (contain == focal_i).astype(np.float32)
    pl_mask = (contain > 0).astype(np.float32)
    pat = np.ascontiguousarray(
        np.tile(np.concatenate([bel_mask, pl_mask])[None, :], (P, 1))
    )

    nc = _get_nc()
    in_maps = [{"pat": pat.copy()} for _ in range(N_CORES)]
    res = run_bass_kernel_spmd(nc, in_maps, list(range(N_CORES)))
    LAST_RESULTS = res

    out_dtype = inputs.dtype
    bel_full = np.empty((B, H, W, NSETS), dtype=out_dtype)
    pl_full = np.empty((B, H, W, NSETS), dtype=out_dtype)
    for b in range(N_CORES):
        bel_full[b] = res.results[b]["bel"].reshape(H, W, NSETS)
        pl_full[b] = res.results[b]["pl"].reshape(H, W, NSETS)
    return (bel_full, pl_full)


# revision 4
# speedup vs baseline: 1.1719x; 1.1719x over previous
"""Trainium2 Bass kernel for nn_BeliefPlausibilityFocused.

reference():
    cardinal_fod = inputs.shape[-1] - 1 = 3; n_sets = 8
    bel[..., j] = 1.0 if (j & focal) == focal else 0.0
    pl[...,  j] = 1.0 if (j & focal) >  0    else 0.0
Both outputs are per-pixel broadcast constants of shape
inputs.shape[:-1] + (8,) = [8, 384, 1248, 8]; the input VALUES are unused.

Strategy (pure data-parallel over batch, per sharding hint):
  - 8 cores, one batch element each. Per-core output: bel/pl each
    [384, 1248, 8] f32 = 15.3 MB -> 30.7 MB HBM writes per core.
  - Host computes the two 8-float masks from `focal` and passes one tiny
    [128, 16] pattern tensor. Device replicates it along the free dim with
    log-doubling DVE copies into two SBUF tiles, then issues large HWDGE
    DMA stores to fill the outputs. Memory-roofline ~= 30.7MB / ~358 GB/s.
"""

import numpy as np

import concourse.bacc as bacc
import concourse.mybir as mybir
import concourse.tile as tile
from concourse.bass_utils import run_bass_kernel_spmd

# Problem shapes (hardcoded per contract: kernel.py must be self-contained).
B, H, W, C = 8, 384, 1248, 4
NSETS = 1 << (C - 1)          # 8
N_CORES = 8
P = 128                        # SBUF partitions

PIX = H * W                    # 479232 pixels per batch element
PER_OUT = PIX * NSETS          # 3,833,856 f32 per output per core
PER_PART = PER_OUT // P        # 29,952 f32 per partition
SRC_F = 1872                   # source tile width; 7488 B per repeat chunk
REP = PER_PART // SRC_F        # 16 stride-0 repeats per store

assert PER_OUT % P == 0 and PER_PART % NSETS == 0 and SRC_F % NSETS == 0
assert SRC_F * REP == PER_PART

_NC_CACHE = {}
LAST_RESULTS = None  # BassKernelResults of the most recent run (for test.py)


def _build_nc():
    """One SPMD program: load 16-float pattern row, replicate, store outputs."""
    nc = bacc.Bacc(None, target_bir_lowering=False)

    pat = nc.dram_tensor("pat", [P, 2 * NSETS], mybir.dt.float32,
                         kind="ExternalInput")
    bel = nc.dram_tensor("bel", [P, PER_PART], mybir.dt.float32,
                         kind="ExternalOutput")
    pl = nc.dram_tensor("pl", [P, PER_PART], mybir.dt.float32,
                        kind="ExternalOutput")

    with tile.TileContext(nc) as tc:
        with tc.tile_pool(name="sbuf", bufs=1) as pool:
            ptile = pool.tile([P, 2 * NSETS], mybir.dt.float32, tag="pt")
            nc.sync.dma_start(out=ptile[:], in_=pat[:])

            belt = pool.tile([P, SRC_F], mybir.dt.float32, tag="belt")
            plt = pool.tile([P, SRC_F], mybir.dt.float32, tag="plt")

            # Fill each small tile with the 8-periodic mask via log-doubling.
            for t, off in ((belt, 0), (plt, NSETS)):
                nc.vector.tensor_copy(out=t[:, 0:NSETS],
                                      in_=ptile[:, off:off + NSETS])
                f = NSETS
                while f < SRC_F:
                    n = min(f, SRC_F - f)
                    nc.vector.tensor_copy(out=t[:, f:f + n], in_=t[:, 0:n])
                    f += n

            # One store per output; the source AP repeats the small tile via
            # a stride-0 dim so each store writes the full 15.3 MB output.
            # bel on the SP HWDGE ring, pl on the ACT HWDGE ring.
            bel3 = bel[:].rearrange("p (r f) -> p r f", r=REP)
            pl3 = pl[:].rearrange("p (r f) -> p r f", r=REP)
            bsrc = belt[:].unsqueeze(1).broadcast_to([P, REP, SRC_F])
            psrc = plt[:].unsqueeze(1).broadcast_to([P, REP, SRC_F])
            nc.sync.dma_start(out=bel3, in_=bsrc)
            nc.scalar.dma_start(out=pl3, in_=psrc)

    nc.finalize()
    return nc


def _get_nc():
    if "nc" not in _NC_CACHE:
        _NC_CACHE["nc"] = _build_nc()
    return _NC_CACHE["nc"]


def kernel(inputs, focal):
    global LAST_RESULTS
    inputs = np.asarray(inputs)
    focal_i = int(np.asarray(focal))
    assert inputs.shape == (B, H, W, C), inputs.shape

    # Host-side mask computation (cheap: 8 elements).
    j = np.arange(NSETS, dtype=np.int64)
    contain = j & focal_i
    bel_mask = (contain == focal_i).astype(np.float32)
    pl_mask = (contain > 0).astype(np.float32)
    pat = np.ascontiguousarray(
        np.tile(np.concatenate([bel_mask, pl_mask])[None, :], (P, 1))
    )

    nc = _get_nc()
    in_maps = [{"pat": pat.copy()} for _ in range(N_CORES)]
    res = run_bass_kernel_spmd(nc, in_maps, list(range(N_CORES)))
    LAST_RESULTS = res

    out_dtype = inputs.dtype
    bel_full = np.empty((B, H, W, NSETS), dtype=out_dtype)
    pl_full = np.empty((B, H, W, NSETS), dtype=out_dtype)
    for b in range(N_CORES):
        bel_full[b] = res.results[b]["bel"].reshape(H, W, NSETS)
        pl_full[b] = res.results[b]["pl"].reshape(H, W, NSETS)
    return (bel_full, pl_full)
